# revision 2
# baseline (speedup 1.0000x reference)
"""Trainium2 Bass kernel for a linear-chain CRF negative log-likelihood.

Problem: S=32768 sequence steps, L=512 tags.
  loss = logsumexp over all paths (forward algorithm) - gold path score.

Algorithm (device):
  In exp-space the forward recurrence is LINEAR: w_{t} = D_t E w_{t-1}
  with E = exp(T) constant and D_t = diag(exp(logit[t])).  Products of
  positive matrices contract to rank-1 at ~0.06/step, so the 32767-step
  serial chain is split into 2048 segments of 16 transitions.  For each
  segment we compute g = M_seg @ 1 (forward chain, all-ones init) and
  h = M_seg^T @ 1 (backward chain).  Host stitches exactly in float64:
      alpha_end = log g + kappa*n + lse(log h + alpha_start) - lse(log g)
  which is exact up to the rank-1 residual (~0.06^16 ~ 1e-20).
  Each of the 8 cores runs its 256 segments as ONE batch: 16 lockstep
  wall-steps of 32 matmuls ([128,128] bf16 blocks of E) + elementwise
  emission multiplies.  The emission table F = exp(logitT) is staged
  step-major so every per-step multiply operand is contiguous bf16.
  The fwd multiply runs on DVE straight out of PSUM; the bwd chain goes
  PSUM -> bf16 SBUF copy on the Activation engine, then a DVE/Pool split
  multiply, so all four compute engines share the elementwise load and
  the tensor engine stays the bottleneck.  Fwd step 0 (state == ones) is
  replaced by a host-precomputed row-sum broadcast, saving 16 matmuls.

  Gold path score (emissions + transitions) is a pure gather: computed
  on host in float64 alongside the stitch.

  Core 7 has 4095 real transitions; one phantom transition (feat=0) pads
  its last segment and is removed exactly in the host stitch using the
  segment's 15-step forward state plus r[i] = lse_j T[j,i].
"""

import numpy as np

import concourse.bass as bass
import concourse.bacc as bacc
import concourse.tile as tile
import concourse.bass_utils as bass_utils
from concourse import mybir

S, L = 32768, 512
NCORES = 8
SPAN = 4096          # transition columns per core (core 7: 4095 real + 1 phantom)
SEG_N = 16           # transitions per segment
SEG_P = 256          # segments per core
KAPPA = 6.74         # constant log-scale folded into E-hat = exp(T - KAPPA)

# bwd elementwise multiply split: DVE does [0:BWD_DVE), Pool the rest
BWD_DVE = 256

F32 = mybir.dt.float32
BF16 = mybir.dt.bfloat16

_CACHE = {}


def _emit_body(tc, io, reps=1, phases=("chain",)):
    nc = tc.nc
    MULT = mybir.AluOpType.mult
    EXP = mybir.ActivationFunctionType.Exp

    import contextlib
    ctx = contextlib.ExitStack()
    const = ctx.enter_context(tc.tile_pool(name="const", bufs=1))
    fin = ctx.enter_context(tc.tile_pool(name="fin", bufs=2))
    fpool = ctx.enter_context(tc.tile_pool(name="fpool", bufs=1))
    us = ctx.enter_context(tc.tile_pool(name="us", bufs=3))
    xs = ctx.enter_context(tc.tile_pool(name="xs", bufs=3))
    xc_pool = ctx.enter_context(tc.tile_pool(name="xc", bufs=3))
    outp = ctx.enter_context(tc.tile_pool(name="outp", bufs=1))
    pf_pool = ctx.enter_context(tc.tile_pool(name="pf", bufs=2, space="PSUM"))
    pb_pool = ctx.enter_context(tc.tile_pool(name="pb", bufs=2, space="PSUM"))

    # ---- constants / weights -------------------------------------------
    kbias = const.tile([128, 1], F32, tag="kbias")
    nc.gpsimd.memset(kbias[:], -KAPPA)
    w_f = []   # fwd lhsT chunks: exp(T^T - k) [i-part, j-free]
    w_b = []   # bwd lhsT chunks: exp(T - k)   [j-part, i-free]
    for c in range(4):
        tt = fin.tile([128, 512], F32, tag="tstage")
        nc.sync.dma_start(tt[:], io["t_tr"][c * 128:(c + 1) * 128, :])
        wf = const.tile([128, 512], BF16, tag=f"wf{c}")
        nc.scalar.activation(wf[:], tt[:], EXP, bias=kbias[:])
        w_f.append(wf)

        tn = fin.tile([128, 512], F32, tag="tstage")
        nc.sync.dma_start(tn[:], io["t_nat"][c * 128:(c + 1) * 128, :])
        wb = const.tile([128, 512], BF16, tag=f"wb{c}")
        nc.scalar.activation(wb[:], tn[:], EXP, bias=kbias[:])
        w_b.append(wb)

    rsum = const.tile([128, 4], F32, tag="rsum")
    nc.sync.dma_start(rsum[:], io["rsum"][:])

    # ---- F = exp(logit - kappa-free), step-major bf16 ------------------
    # f_all[:, s*1024 + c*256 + k] = exp(logit[t0 + 16k + s, 128c + p])
    f_all = fpool.tile([128, 16 * 1024], BF16, tag="f_all")
    for q in range(4):
        chunk = fin.tile([128, 4096], F32, tag="fstage")
        nc.sync.dma_start(chunk[:], io["f_pre"][:, q * 4096:(q + 1) * 4096])
        nc.scalar.activation(f_all[:, q * 4096:(q + 1) * 4096], chunk[:], EXP)

    def fstep(s):
        return f_all[:, s * 1024:(s + 1) * 1024]

    for rep in range(reps):
        if "chain" not in phases:
            break
        # fwd state u_m = state after m transitions; u_1 via rsum shortcut
        u = us.tile([128, 1024], BF16, tag="u")
        for jc in range(4):
            nc.vector.tensor_scalar(
                u[:, jc * 256:(jc + 1) * 256],
                fstep(0)[:, jc * 256:(jc + 1) * 256],
                rsum[:, jc:jc + 1], None, op0=MULT)

        xin = fstep(15)  # bwd x_0 = f(15), used directly as matmul rhs
        for s in range(SEG_N):
            # bwd: psum_b[ic*256+k] = sum_jc Wb[jc][:,ic]^T @ x[jc*256+k]
            psum_b = pb_pool.tile([128, 1024], F32, tag="pb")
            for ic in range(4):
                for jc in range(4):
                    nc.tensor.matmul(
                        psum_b[:, ic * 256:(ic + 1) * 256],
                        w_b[jc][:, ic * 128:(ic + 1) * 128],
                        xin[:, jc * 256:(jc + 1) * 256],
                        start=(jc == 0), stop=(jc == 3))
            # fwd: psum_f[jc*256+k] = sum_ic Wf[ic][:,jc]^T @ u[ic*256+k]
            if s < SEG_N - 1:
                psum_f = pf_pool.tile([128, 1024], F32, tag="pf")
                for jc in range(4):
                    for ic in range(4):
                        nc.tensor.matmul(
                            psum_f[:, jc * 256:(jc + 1) * 256],
                            w_f[ic][:, jc * 128:(jc + 1) * 128],
                            u[:, ic * 256:(ic + 1) * 256],
                            start=(ic == 0), stop=(ic == 3))

            if s < SEG_N - 1:
                # bwd elementwise: x_{s+1} = (Act copy of psum_b) * f(14-s)
                xcp = xc_pool.tile([128, 1024], BF16, tag="xc")
                nc.scalar.copy(xcp[:], psum_b[:])
                xnew = xs.tile([128, 1024], BF16, tag="x")
                fb = fstep(SEG_N - 2 - s)
                nc.vector.tensor_mul(
                    xnew[:, 0:BWD_DVE], xcp[:, 0:BWD_DVE], fb[:, 0:BWD_DVE])
                nc.gpsimd.tensor_mul(
                    xnew[:, BWD_DVE:], xcp[:, BWD_DVE:], fb[:, BWD_DVE:])
                xin = xnew
            else:
                h_sb = outp.tile([128, 1024], BF16, tag="h_sb")
                nc.scalar.copy(h_sb[:], psum_b[:])
                nc.sync.dma_start(io["h_out"][:], h_sb[:])

            if s < SEG_N - 1:
                # fwd elementwise: u_{s+2} = psum_f * f(s+1), straight
                # from PSUM on DVE
                unew = us.tile([128, 1024], BF16, tag="u")
                nc.vector.tensor_mul(unew[:], psum_f[:], fstep(s + 1))
                u = unew
                if s + 2 == SEG_N - 1:
                    nc.sync.dma_start(io["gp_out"][:], u[:])
        nc.sync.dma_start(io["g_out"][:], u[:])

    ctx.close()


def build_program(reps=1, phases=("chain",)):
    nc = bacc.Bacc("TRN2", target_bir_lowering=False, debug=False,
                   num_devices=NCORES)
    io = {}
    def inp(name, shape, dt=F32):
        io[name] = nc.dram_tensor(name, shape, dt, kind="ExternalInput").ap()
    def outp(name, shape, dt):
        io[name] = nc.dram_tensor(name, shape, dt, kind="ExternalOutput").ap()

    inp("f_pre", [128, 16 * 1024])
    inp("t_nat", [L, L])
    inp("t_tr", [L, L])
    inp("rsum", [128, 4])
    outp("g_out", [128, 1024], BF16)
    outp("gp_out", [128, 1024], BF16)
    outp("h_out", [128, 1024], BF16)

    with tile.TileContext(nc) as tc:
        _emit_body(tc, io, reps=reps, phases=phases)
    nc.compile()
    return nc


def make_in_maps(logit, labels, T):
    """Host-side sharding/layout prep. logit [S,L] f32, labels [S] int, T [L,L] f32."""
    logit = np.asarray(logit, dtype=np.float32)
    T = np.asarray(T, dtype=np.float32)

    logitT_full = np.ascontiguousarray(logit.T)          # [L, S]
    t_nat = np.ascontiguousarray(T)
    t_tr = np.ascontiguousarray(T.T)
    rsum = np.exp(np.asarray(T, dtype=np.float64) - KAPPA).sum(axis=1)
    rsum = np.ascontiguousarray(
        rsum.reshape(4, 128).T.astype(np.float32))       # [p, jc]

    in_maps = []
    for c in range(NCORES):
        t0 = c * SPAN + 1                     # first transition of this core
        sl = np.zeros((L, SPAN), dtype=np.float32)
        n_real = min(SPAN, S - t0)            # 4096, core 7: 4095
        sl[:, :n_real] = logitT_full[:, t0:t0 + n_real]
        # [c, p, k, s] -> [p, s, c, k]
        f_pre = np.ascontiguousarray(
            sl.reshape(4, 128, SEG_P, SEG_N).transpose(1, 3, 0, 2)
            .reshape(128, SEG_N * 1024))
        in_maps.append({
            "f_pre": f_pre,
            "t_nat": t_nat,
            "t_tr": t_tr,
            "rsum": rsum,
        })
    return in_maps


def _lse(x, axis=None):
    m = np.max(x, axis=axis, keepdims=True)
    out = m + np.log(np.sum(np.exp(x - m), axis=axis, keepdims=True))
    return np.squeeze(out, axis=axis) if axis is not None else out.reshape(())


def host_stitch(results, logit, labels, T):
    """Combine per-core segment chain outputs into the scalar loss (float64)."""
    logit64 = np.asarray(logit, dtype=np.float64)
    T64 = np.asarray(T, dtype=np.float64)
    labels = np.asarray(labels).astype(np.int64)

    def vecs(arr):
        # [128, 1024] bf16 -> [512, 256] float64 (tag, segment)
        a = np.asarray(arr).astype(np.float64).reshape(128, 4, SEG_P)
        return a.transpose(1, 0, 2).reshape(L, SEG_P)

    r_corr = None
    with np.errstate(divide="ignore"):
        alpha = logit64[0].copy()
        for c in range(NCORES):
            g = np.log(vecs(results[c]["g_out"]))
            gp = np.log(vecs(results[c]["gp_out"]))
            h = np.log(vecs(results[c]["h_out"]))
            for k in range(SEG_P):
                phantom = (c == NCORES - 1 and k == SEG_P - 1)
                if not phantom:
                    alpha = (g[:, k] + KAPPA * SEG_N
                             + _lse(h[:, k] + alpha) - _lse(g[:, k]))
                else:
                    if r_corr is None:
                        r_corr = _lse(T64, axis=0)   # r[i] = lse_j T[j,i]
                    alpha = (gp[:, k] + KAPPA * SEG_N
                             + _lse(h[:, k] + alpha) - _lse(gp[:, k] + r_corr))
        log_z = _lse(alpha)

    # gold path score, exactly, on host
    gold = (float(logit64[0, labels[0]])
            + float(logit64[np.arange(1, S), labels[1:]].sum())
            + float(T64[labels[1:], labels[:-1]].sum()))
    return float(log_z) - gold


def kernel(logit, labels, T):
    key = "prog"
    if key not in _CACHE:
        _CACHE[key] = build_program()
    nc = _CACHE[key]
    in_maps = make_in_maps(logit, labels, T)
    res = bass_utils.run_bass_kernel_spmd(nc, in_maps, core_ids=list(range(NCORES)))
    loss = host_stitch(res.results, logit, labels, T)
    return np.array(loss, dtype=np.float32)
